# revision 8
# baseline (speedup 1.0000x reference)
"""Trainium2 Bass kernel for nn_DenseCRFFDR (dense CRF mean-field, 2 labels).

Self-contained: hardcodes shapes/sharding. kernel(**inputs) takes FULL
numpy inputs and returns the FULL output tuple (h_new, f_1).

Algorithm (validated in numpy): with 2 labels, softmax == sigmoid(diff) and
q1 = 1-q0, so the whole 5-iteration CRF collapses to a recursion on a single
map:  diff <- Cd - ad*blur3d(q0) - bd*rden*num_raw(q0),  q0 = sigmoid(diff).
All 2x2 matrix algebra folds into host scalars (ad, bd, A1, B1) baked into
PE stationary matrices; the bilateral's H-axis (partition) shifts are done by
shift-matrix matmuls on the TensorEngine, which also performs all 27-term
accumulations in PSUM (identity/banded/shift stationaries). The DVE only does
the 27 Hadamard products per iteration (fp16, 2x mode).

Sharding: D=64 split across 8 cores (8 planes each + 2-plane halo); per-
iteration halo exchange of q0 via AllGather of boundary planes, neighbor
slices addressed dynamically with partition_id; edge cores mask wrapped data.
"""
import math
import numpy as np

# ---------------- problem constants (hardcoded per contract) ----------------
D, H, W = 64, 128, 128
N_CORES = 8
DL = D // N_CORES            # 8 own planes per core
HALO = 2
NP = DL + 2 * HALO           # 12 plane slots
WS = 136                     # stored width; valid cols [4, 132)
CV = 4                       # first valid col
NUM_ITERS = 5
SIGMA_SPATIAL_BLUR = 1.0
BLUR_RADIUS = 2
S2R = 2.0 * 0.5 ** 2         # 2*sigma_range^2 = 0.5
S2S = 2.0 * 1.5 ** 2         # 2*sigma_spatial^2 = 4.5
LOG_SQRT_2PI = 0.5 * float(np.log(2.0 * np.pi))

_offs = np.arange(-BLUR_RADIUS, BLUR_RADIUS + 1)
_k1d = np.exp(-_offs.astype(np.float64) ** 2 / (2.0 * SIGMA_SPATIAL_BLUR ** 2))
K1D = (_k1d / _k1d.sum()).astype(np.float32)

OFFSETS = [(dz, dy, dx) for dz in (-1, 0, 1) for dy in (-1, 0, 1)
           for dx in (-1, 0, 1)]
# mirrored dy=0 offsets are derived from their negatives via shifted reads
MIRRORED = [s for s in OFFSETS if s[1] == 0 and s < (0, 0, 0)]
COMPUTED = [s for s in OFFSETS if not (s[1] == 0 and s < (0, 0, 0))]

DEBUG = 0      # 0: none, 1: dump precompute intermediates + diff1

_CACHE = {}


# ---------------- host-side helpers ----------------
def _gmat(s):
    dz, dy, dx = s
    return math.exp(-(dz * dz + dy * dy + dx * dx) / S2S)


def _shift_mat(dy, val=1.0):
    # (S @ v)[m] = val * v[m+dy]
    S = np.zeros((H, H), np.float32)
    for m in range(H):
        k = m + dy
        if 0 <= k < H:
            S[m, k] = val
    return S


def _banded_bh(scale=1.0):
    Bh = np.zeros((H, H), np.float32)
    for m in range(H):
        for t in range(-BLUR_RADIUS, BLUR_RADIUS + 1):
            k = m + t
            if 0 <= k < H:
                Bh[m, k] = scale * K1D[t + BLUR_RADIUS]
    return Bh


def _host_scalars(spatial_w, bilateral_w, compat):
    CS = (compat @ spatial_w).astype(np.float64)
    CB = (compat @ bilateral_w).astype(np.float64)
    A = CS[0] - CS[1]
    B = CB[0] - CB[1]
    ad = float(A[0] - A[1])
    bd = float(B[0] - B[1])
    return float(A[1]), float(B[1]), ad, bd


def _build_mats16(A1, B1, ad, bd):
    """The 22 fp16 stationary matrices, order fixed:
    0: I (=SD0), 1: SD-1, 2: SD+1,
    3..7:  cI tap dx=-2..2           (K1D[t] * I)
    8..12: DH tap dz=-2..2           (-ad*K1D[t] * Bh)
    13..17: DH1 tap dz=-2..2         (-A1*K1D[t] * Bh)
    18..20: SB dy=-1,0,1             (-bd * S_dy)
    (ones-pass SB1 dy variants at 21..23) -> total 24
    """
    mats = []
    mats.append(np.eye(H, dtype=np.float32))            # 0
    mats.append(_shift_mat(-1))                         # 1
    mats.append(_shift_mat(+1))                         # 2
    for t in range(5):
        mats.append(K1D[t] * np.eye(H, dtype=np.float32))      # 3..7
    for t in range(5):
        mats.append(_banded_bh(-ad * K1D[t]))                  # 8..12
    for t in range(5):
        mats.append(_banded_bh(-A1 * K1D[t]))                  # 13..17
    for dy in (-1, 0, 1):
        mats.append(_shift_mat(dy, -bd))                       # 18..20
    for dy in (-1, 0, 1):
        mats.append(_shift_mat(dy, -B1))                       # 21..23
    # matmul computes lhsT.T @ rhs -> pass transposes as the stationaries
    return np.stack([m.T for m in mats]).astype(np.float16)


M_I, M_SDM, M_SDP = 0, 1, 2
M_CI = 3      # +t
M_DH = 8      # +t
M_DH1 = 13    # +t
M_SB = 18     # +dy+1
M_SB1 = 21    # +dy+1


def _vslab(x, base, lo, hi, dtype):
    """volume planes [base+lo, base+hi) of (D,H,W) x -> [H, hi-lo, WS],
    zero padded OOB planes and W pad cols."""
    n = hi - lo
    out = np.zeros((H, n, WS), dtype)
    for i, d in enumerate(range(base + lo, base + hi)):
        if 0 <= d < D:
            out[:, i, CV:CV + W] = x[d].astype(dtype)
    return out


def _yshift(t, dy):
    """rows: out[j] = t[j-dy]"""
    out = np.zeros_like(t)
    if dy == 0:
        out[:] = t
    elif dy == 1:
        out[1:] = t[:-1]
    else:
        out[:-1] = t[1:]
    return out


def _xshift1(t):
    """cols: out[c] = t[c+1]"""
    out = np.zeros_like(t)
    out[:, :, :-1] = t[:, :, 1:]
    return out


def _build_core_inputs(image, h, f_1, w_0, A1, B1, ad, bd):
    I = np.asarray(image[0, 0], np.float32)
    h0 = np.asarray(h[0, 0], np.float32)
    h1 = np.asarray(h[0, 1], np.float32)
    f = np.asarray(f_1[0, 0], np.float32)
    w0 = float(np.asarray(w_0)[0])

    logprob = -0.5 * I * I - LOG_SQRT_2PI
    Ud_full = (h0 - h1) * (-(w0 + logprob - f))
    hd = (h0 - h1).astype(np.float32)
    q0_full = (1.0 / (1.0 + np.exp(-hd))).astype(np.float32)

    gsum = sum(math.exp(-(dz * dz + 1 + dx * dx) / S2S)
               for dz in (-1, 0, 1) for dx in (-1, 0, 1))
    gvec = np.zeros((H, 1), np.float32)
    gvec[0, 0] = gsum
    gvec[H - 1, 0] = gsum

    mats16 = _build_mats16(A1, B1, ad, bd)
    i32 = np.eye(H, dtype=np.float32)
    # ln(g) biases for the rho' exp, indexed by squared offset distance d2
    lng = np.zeros((H, 4), np.float32)
    for d2 in range(4):
        lng[:, d2] = -d2 / S2S

    in_maps = []
    for c in range(N_CORES):
        base = c * DL
        img_y0 = _vslab(I, base, -HALO, DL + HALO, np.float16)
        q0i = _vslab(q0_full, base, -HALO, DL + HALO, np.float16)
        onesq = _vslab(np.ones_like(I), base, -HALO, DL + HALO, np.float16)
        mask = np.zeros((H, 4, WS), np.float16)
        if c > 0:
            mask[:, 0:2, CV:CV + W] = 1.0
        if c < N_CORES - 1:
            mask[:, 2:4, CV:CV + W] = 1.0
        in_maps.append({
            "mats16": mats16,
            "i32": i32,
            "img_y0": img_y0,
            "img_ym": _yshift(img_y0, -1),
            "img_yp": _yshift(img_y0, +1),
            "img_x1": _xshift1(img_y0),
            "ud": _vslab(Ud_full, base, 0, DL, np.float32)[:, :, CV:CV + W].copy(),
            "q0i": q0i,
            "q1i": _xshift1(q0i),
            "onesq": onesq,
            "mask": mask,
            "gvec": gvec,
            "lng": lng,
        })
    return in_maps


# ---------------- bass program ----------------
def _build_nc():
    import concourse.bass as bass
    import concourse.tile as tile
    from concourse import bacc, mybir
    from contextlib import ExitStack

    dt16 = mybir.dt.float16
    dt32 = mybir.dt.float32
    AF = mybir.ActivationFunctionType
    OP = mybir.AluOpType

    nc = bacc.Bacc("TRN2", target_bir_lowering=False, debug=False,
                   num_devices=N_CORES)

    # I/O
    din = {}
    din["mats16"] = nc.dram_tensor("mats16", [24, H, H], dt16,
                                   kind="ExternalInput").ap()
    din["i32"] = nc.dram_tensor("i32", [H, H], dt32, kind="ExternalInput").ap()
    for nm in ("img_y0", "img_ym", "img_yp", "img_x1", "q0i", "q1i", "onesq"):
        din[nm] = nc.dram_tensor(nm, [H, NP, WS], dt16,
                                 kind="ExternalInput").ap()
    din["ud"] = nc.dram_tensor("ud", [H, DL, W], dt32,
                               kind="ExternalInput").ap()
    din["mask"] = nc.dram_tensor("mask", [H, 4, WS], dt16,
                                 kind="ExternalInput").ap()
    din["gvec"] = nc.dram_tensor("gvec", [H, 1], dt32,
                                 kind="ExternalInput").ap()
    din["lng"] = nc.dram_tensor("lng", [H, 4], dt32,
                                kind="ExternalInput").ap()
    h_out = nc.dram_tensor("h_out", [2, DL, H, W], dt32,
                           kind="ExternalOutput").ap()
    dbg = {}
    if DEBUG:
        for nm, shp, dt_ in [("dbg_rho", [H, 10, 132], dt16),
                             ("dbg_den", [H, DL, W], dt16),
                             ("dbg_rden", [H, DL, W], dt16),
                             ("dbg_wpp", [H, DL, W], dt16),
                             ("dbg_cd", [H, DL, W], dt32),
                             ("dbg_diff1", [H, DL, W], dt32)]:
            dbg[nm] = nc.dram_tensor(nm, shp, dt_, kind="ExternalOutput").ap()

    with tile.TileContext(nc) as tc, ExitStack() as ctx:
        consts = ctx.enter_context(tc.tile_pool(name="consts", bufs=1))
        work = ctx.enter_context(tc.tile_pool(name="work", bufs=1))
        vpool = ctx.enter_context(tc.tile_pool(name="vpool", bufs=4))
        tmp = ctx.enter_context(tc.tile_pool(name="tmp", bufs=2))
        dbgp = ctx.enter_context(tc.tile_pool(name="dbgp", bufs=1))
        psum = ctx.enter_context(
            tc.tile_pool(name="psum", bufs=2, space="PSUM"))
        psum_t1 = ctx.enter_context(
            tc.tile_pool(name="psum_t1", bufs=1, space="PSUM"))
        dram = ctx.enter_context(tc.tile_pool(name="dram", bufs=2,
                                              space="DRAM"))

        # ---- load constants ----
        mats = consts.tile([H, 24, H], dt16)
        for i in range(24):
            nc.sync.dma_start(mats[:, i, :], din["mats16"][i, :, :])
        i32t = consts.tile([H, H], dt32)
        nc.sync.dma_start(i32t[:], din["i32"][:])
        maskt = consts.tile([H, 4, WS], dt16)
        nc.sync.dma_start(maskt[:], din["mask"][:])
        gvect = consts.tile([H, 1], dt32)
        nc.sync.dma_start(gvect[:], din["gvec"][:])
        lngt = consts.tile([H, 4], dt32)
        nc.sync.dma_start(lngt[:], din["lng"][:])
        udt = consts.tile([H, DL, W], dt32)
        nc.sync.dma_start(udt[:], din["ud"][:])

        def mat(i):
            return mats[:, i, :]

        # ---- persistent work tiles ----
        q0 = work.tile([H, NP, WS], dt16)
        q1 = work.tile([H, NP, WS], dt16)
        onq = work.tile([H, NP, WS], dt16)
        nc.sync.dma_start(q0[:], din["q0i"][:])
        nc.sync.dma_start(q1[:], din["q1i"][:])
        nc.sync.dma_start(onq[:], din["onesq"][:])
        t1sb = work.tile([H, NP, W], dt16)
        cdt = work.tile([H, DL, W], dt32)
        rden = work.tile([H, DL, W], dt16)
        rden_m = work.tile([H, DL, W], dt16)   # rden_y[-1][j] = rden[j+1]
        rden_p = work.tile([H, DL, W], dt16)   # rden_y[+1][j] = rden[j-1]
        wpp = {s: work.tile([H, DL, W], dt16, name=f"wpp_{i}")
               for i, s in enumerate(OFFSETS)}

        # =========== PHASE 1: rho', den, rden, W'' ===========
        with ExitStack() as pctx:
            imgs = pctx.enter_context(tc.tile_pool(name="imgs", bufs=1))
            rhop = pctx.enter_context(tc.tile_pool(name="rhop", bufs=1))

            img = {}
            for nm in ("img_y0", "img_ym", "img_yp", "img_x1"):
                t = imgs.tile([H, NP, WS], dt16, name=nm)
                nc.sync.dma_start(t[:], din[nm][:])
                img[nm] = t
            img_y = {-1: img["img_ym"], 0: img["img_y0"], 1: img["img_yp"]}

            rho = {}
            for s in COMPUTED:
                dz, dy, dx = s
                # delta = img_y[dy][:,1:11,2:134] - img_(z,x)-shifted
                ia = img_y[dy][:, 1:11, 2:134]
                if dx % 2 == 0:
                    ib = img["img_y0"][:, 1 + dz:11 + dz, 2 + dx:134 + dx]
                else:
                    ib = img["img_x1"][:, 1 + dz:11 + dz, 1 + dx:133 + dx]
                dlt = tmp.tile([H, 10, 132], dt16, tag="delta")
                nc.vector.tensor_tensor(dlt[:], ia, ib, OP.subtract)
                sq = tmp.tile([H, 10, 132], dt16, tag="sq")
                nc.vector.tensor_tensor(sq[:], dlt[:], dlt[:], OP.mult)
                r = rhop.tile([H, 10, 132], dt16, name=f"rho_{s}")
                d2 = dz * dz + dy * dy + dx * dx
                nc.scalar.activation(r[:], sq[:], AF.Exp,
                                     scale=-1.0 / S2R,
                                     bias=lngt[:, d2:d2 + 1])
                rho[s] = (r, 0, 0)  # tile, plane shift, col shift
            for s in MIRRORED:
                dz, dy, dx = s
                src, _, _ = rho[(-dz, 0, -dx)]
                rho[s] = (src, dz, dx)

            # den: psum[m] = sum_s rho_s[m+dy]  (SD matmuls), 2 halves
            ps_den = psum.tile([H, DL, W], dt32, tag="ps")
            for si, s in enumerate(OFFSETS):
                dz, dy, dx = s
                r, pz, px = rho[s]
                sd = {-1: M_SDM, 0: M_I, 1: M_SDP}[dy]
                for hf in range(2):
                    rhs = r[:, 1 + pz + hf * 4: 5 + pz + hf * 4,
                            2 + px: 130 + px]
                    nc.tensor.matmul(ps_den[:, hf * 4:hf * 4 + 4, :],
                                     mat(sd), rhs,
                                     start=(si == 0), stop=(si == 26))

            # E = exp(-I^2/S2R) on own planes; den += gvec*E ; rden = 1/den
            isq = tmp.tile([H, DL, W], dt16, tag="scr8")
            nc.vector.tensor_tensor(isq[:],
                                    img["img_y0"][:, 2:10, 4:132],
                                    img["img_y0"][:, 2:10, 4:132], OP.mult)
            emap = tmp.tile([H, DL, W], dt16, tag="scr8")
            nc.scalar.activation(emap[:], isq[:], AF.Exp, scale=-1.0 / S2R)
            den_sb = tmp.tile([H, DL, W], dt16, tag="scr8")
            nc.vector.scalar_tensor_tensor(den_sb[:], emap[:], gvect[:],
                                           ps_den[:], OP.mult, OP.add)
            with nc.allow_low_precision(reason="den in [1,27]; fp16 ok"):
                nc.vector.reciprocal(rden[:], den_sb[:])

            # rden_y variants via PE shifts
            ps_r = psum.tile([H, DL, W], dt32, tag="ps")
            for hf in range(2):
                nc.tensor.matmul(ps_r[:, hf * 4:hf * 4 + 4, :], mat(M_SDP),
                                 rden[:, hf * 4:hf * 4 + 4, :],
                                 start=(hf == 0) or True, stop=True)
            nc.scalar.copy(rden_m[:], ps_r[:])
            ps_r2 = psum.tile([H, DL, W], dt32, tag="ps")
            for hf in range(2):
                nc.tensor.matmul(ps_r2[:, hf * 4:hf * 4 + 4, :], mat(M_SDM),
                                 rden[:, hf * 4:hf * 4 + 4, :],
                                 start=True, stop=True)
            nc.scalar.copy(rden_p[:], ps_r2[:])
            rden_y = {-1: rden_m, 0: rden, 1: rden_p}

            # W''_s = rho'_s[own planes] * rden_y[dy]
            for s in OFFSETS:
                dz, dy, dx = s
                r, pz, px = rho[s]
                nc.vector.tensor_tensor(
                    wpp[s][:], r[:, 1 + pz:9 + pz, 2 + px:130 + px],
                    rden_y[dy][:], OP.mult)

            if DEBUG:
                nc.sync.dma_start(dbg["dbg_rho"][:], rho[(0, 0, 1)][0][:])
                nc.sync.dma_start(dbg["dbg_den"][:], den_sb[:])
                nc.sync.dma_start(dbg["dbg_rden"][:], rden[:])
                nc.sync.dma_start(dbg["dbg_wpp"][:], wpp[(1, 1, 1)][:])

        # =========== shared pipeline: one CRF "message pass" ===========
        def message_pass(ps2, qa, qb, i_dh, i_sb, with_cd):
            """ps2 (psum [H, DL, W]) = [Cd +] DH-blur + bilateral of (qa,qb).
            qa: q-field [H,NP,WS], qb: its +1-col-shifted copy."""
            # Cd add (fp32) opens the accumulation group
            for hf in range(2):
                nc.tensor.matmul(ps2[:, hf * 4:hf * 4 + 4, :], i32t[:],
                                 cdt[:, hf * 4:hf * 4 + 4, :],
                                 start=True, stop=False)
            # W-blur taps -> t1 psum (3 banks of 4 planes)
            t1ps = psum_t1.tile([H, NP, W], dt32, tag="t1")
            for ti, dx in enumerate(range(-2, 3)):
                for third in range(3):
                    sl = slice(third * 4, third * 4 + 4)
                    if dx % 2 == 0:
                        rhs = qa[:, sl, CV + dx:CV + W + dx]
                    else:
                        rhs = qb[:, sl, CV - 1 + dx:CV - 1 + W + dx]
                    nc.tensor.matmul(t1ps[:, sl, :], mat(M_CI + ti), rhs,
                                     start=(ti == 0), stop=(ti == 4))
            nc.scalar.copy(t1sb[:], t1ps[:])
            # DH taps -> ps2
            for ti, dz in enumerate(range(-2, 3)):
                for hf in range(2):
                    rhs = t1sb[:, 2 + hf * 4 + dz:6 + hf * 4 + dz, :]
                    nc.tensor.matmul(ps2[:, hf * 4:hf * 4 + 4, :],
                                     mat(i_dh + ti), rhs,
                                     start=False, stop=False)
            # bilateral: products on DVE, shift-accumulate on PE
            for si, s in enumerate(OFFSETS):
                dz, dy, dx = s
                if dx % 2 == 0:
                    qr = qa[:, 2 + dz:10 + dz, CV + dx:CV + W + dx]
                else:
                    qr = qb[:, 2 + dz:10 + dz, CV - 1 + dx:CV - 1 + W + dx]
                v = vpool.tile([H, DL, W], dt16, tag="v")
                nc.vector.tensor_tensor(v[:], qr, wpp[s][:], OP.mult)
                sb = i_sb + dy + 1
                for hf in range(2):
                    nc.tensor.matmul(ps2[:, hf * 4:hf * 4 + 4, :], mat(sb),
                                     v[:, hf * 4:hf * 4 + 4, :],
                                     start=False,
                                     stop=(si == 26 and hf == 1))

        # =========== PHASE 2: ones-pass -> Cd ===========
        # Cd tile must exist before message_pass reads it; for the ones pass
        # we initialize cdt with Ud (the rest of Cd sums into it after).
        nc.vector.tensor_copy(cdt[:], udt[:])
        # ones pass without the Cd add: use a zeroed... simplest: run with
        # with_cd=True reading cdt=Ud, so ps_ones = Ud + (-A1 G1) + (-B1 rden W1)
        # which IS Cd. Then copy psum -> cdt.
        on1 = work.tile([H, NP, WS], dt16)   # ones shifted copy (cols +1)
        # build on1 = x-shift of onesq on device: copy cols 1..WS
        nc.vector.tensor_copy(on1[:, :, 0:WS - 1], onq[:, :, 1:WS])
        nc.vector.memset(on1[:, :, WS - 1:WS], 0)
        ps_ones = psum.tile([H, DL, W], dt32, tag="ps")
        message_pass(ps_ones, onq, on1, M_DH1, M_SB1, True)
        nc.vector.tensor_copy(cdt[:], ps_ones[:])

        if DEBUG:
            nc.sync.dma_start(dbg["dbg_cd"][:], cdt[:])

        # =========== PHASE 3: iterations ===========
        rank = nc.partition_id()
        lo_r = (rank + N_CORES - 1) % N_CORES
        hi_r = (rank + 1) % N_CORES

        ps2 = None
        for it in range(NUM_ITERS):
            if it > 0:
                # sigmoid of previous psum -> q0 own planes (+ q1 copy)
                nc.scalar.activation(q0[:, 2:10, CV:CV + W], ps2[:],
                                     AF.Sigmoid)
                nc.scalar.activation(q1[:, 2:10, CV - 1:CV - 1 + W], ps2[:],
                                     AF.Sigmoid)
                # halo exchange via AllGather of boundary planes
                cc_in = dram.tile([H, 4, WS], dt16, tag="cc_in")
                cc_out = dram.tile([N_CORES, H, 4, WS], dt16,
                                   addr_space="Shared", tag="cc_out")
                nc.sync.dma_start(cc_in[:, 0:2, :], q0[:, 2:4, :])
                nc.sync.dma_start(cc_in[:, 2:4, :], q0[:, 8:10, :])
                import concourse.mybir as mybir_  # local alias
                nc.gpsimd.collective_compute(
                    "AllGather", OP.bypass,
                    replica_groups=[list(range(N_CORES))],
                    ins=[cc_in[:].opt()], outs=[cc_out[:].opt()])
                stage = tmp.tile([H, 4, WS], dt16, tag="stage")
                import concourse.bass as bass_
                nc.sync.dma_start(stage[:, 0:2, :],
                                  cc_out[bass_.ds(lo_r, 1), :, 2:4, :])
                nc.sync.dma_start(stage[:, 2:4, :],
                                  cc_out[bass_.ds(hi_r, 1), :, 0:2, :])
                # masked halo writes (q0) + shifted masked writes (q1)
                nc.vector.tensor_tensor(q0[:, 0:2, :], stage[:, 0:2, :],
                                        maskt[:, 0:2, :], OP.mult)
                nc.vector.tensor_tensor(q0[:, 10:12, :], stage[:, 2:4, :],
                                        maskt[:, 2:4, :], OP.mult)
                nc.vector.tensor_tensor(q1[:, 0:2, 0:WS - 1],
                                        stage[:, 0:2, 1:WS],
                                        maskt[:, 0:2, 1:WS], OP.mult)
                nc.vector.tensor_tensor(q1[:, 10:12, 0:WS - 1],
                                        stage[:, 2:4, 1:WS],
                                        maskt[:, 2:4, 1:WS], OP.mult)
            ps2 = psum.tile([H, DL, W], dt32, tag="ps")
            message_pass(ps2, q0, q1, M_DH, M_SB, True)
            if DEBUG and it == 0:
                dd = dbgp.tile([H, DL, W], dt32, tag="dbgdiff")
                nc.vector.tensor_copy(dd[:], ps2[:])
                nc.sync.dma_start(dbg["dbg_diff1"][:], dd[:])


        # =========== output ===========
        h0t = work.tile([H, DL, W], dt32)
        nc.scalar.activation(h0t[:], ps2[:], AF.Sigmoid)
        h1t = work.tile([H, DL, W], dt32)
        nc.vector.tensor_scalar(h1t[:], h0t[:], -1.0, 1.0, OP.mult, OP.add)
        for d in range(DL):
            nc.sync.dma_start(h_out[0, d, :, :], h0t[:, d, :])
            nc.sync.dma_start(h_out[1, d, :, :], h1t[:, d, :])

    nc.compile()
    return nc


def _get_nc():
    if "nc" not in _CACHE:
        _CACHE["nc"] = _build_nc()
    return _CACHE["nc"]


def kernel(image, h, f_1, w_0, spatial_ker_weights, bilateral_ker_weights,
           compatibility_matrix):
    from concourse.bass_utils import run_bass_kernel_spmd

    A1, B1, ad, bd = _host_scalars(
        np.asarray(spatial_ker_weights, np.float64),
        np.asarray(bilateral_ker_weights, np.float64),
        np.asarray(compatibility_matrix, np.float64))
    in_maps = _build_core_inputs(np.asarray(image), np.asarray(h),
                                 np.asarray(f_1), np.asarray(w_0),
                                 A1, B1, ad, bd)
    nc = _get_nc()
    res = run_bass_kernel_spmd(nc, in_maps, list(range(N_CORES)))
    _CACHE["last_results"] = res

    h_new = np.empty((1, 2, D, H, W), np.float32)
    for c in range(N_CORES):
        out = res.results[c]["h_out"]          # [2, DL, H, W]
        h_new[0, :, c * DL:(c + 1) * DL] = out
    return (h_new, np.asarray(f_1))
